# revision 1
# baseline (speedup 1.0000x reference)
"""Trainium2 Bass kernel for nn_NonLocalBlock (multi-head non-local attention
block with conv/BN/SE tail).

Sharding: 8 cores = 2 batches x 4 query(o)-slices of 1024. Each core computes
full attention (all 4 heads, full key length 4096) for its o-slice, the conv
stack on its slice, and joins the SE squeeze via a 4-core AllGather.

Math notes:
 - softmax(x) computed as exp(x/8) normalized AFTER the PV matmul: an extra
   ones-row appended to V^T gives the row sums in the same matmul (M=65).
 - exp is safe un-maxed: logits are O(5), fp32 exp handles it.
 - conv bias bv folds out: message = M/s + bv (softmax weights sum to 1), so
   x = (feat - bv) - M*r with (feat - bv) precomputed on host.
 - BN is inference-mode: host folds to per-channel scale/shift.
"""
import numpy as np
import ml_dtypes

import concourse.bass as bass
import concourse.tile as tile
from concourse import bacc, mybir
from concourse.bass_utils import run_bass_kernel_spmd

FP32 = mybir.dt.float32
BF16 = mybir.dt.bfloat16
AX = mybir.AxisListType
ALU = mybir.AluOpType
ACTF = mybir.ActivationFunctionType

C, CH, N, BS, HEADS, DH = 256, 128, 4096, 2, 4, 64
O = 1024          # per-core o-slice
OC = 512          # o-chunk
NT = N // 128     # 32 i-tiles
EPS = 1e-5

_CACHE = {}


def _build(dbg=False):
    nc = bacc.Bacc(None, target_bir_lowering=False, debug=False)

    di = {}
    def inp(name, shape, dt):
        di[name] = nc.dram_tensor(name, list(shape), dt, kind="ExternalInput")
        return di[name]

    feat_bf = inp("feat_bf", [C, N], BF16)
    feat_q  = inp("feat_q", [C, O], BF16)
    feat_res = inp("feat_res", [C, O], FP32)
    feat_bv4 = inp("feat_bv4", [64, HEADS * O], FP32)
    wq_t = inp("wq_t", [C, C], BF16)
    wk_t = inp("wk_t", [C, C], BF16)
    wv_t = inp("wv_t", [C, C], BF16)
    bq2 = inp("bq2", [128, 2], FP32)
    bk2 = inp("bk2", [128, 2], FP32)
    w1_t = inp("w1_t", [C, CH], BF16)
    w2_t = inp("w2_t", [CH, CH], BF16)
    w3_t = inp("w3_t", [CH, C], BF16)
    bn1_s = inp("bn1_s", [128, 1], FP32)
    bn1_b = inp("bn1_b", [128, 1], FP32)
    bn2_s = inp("bn2_s", [128, 1], FP32)
    bn2_b = inp("bn2_b", [128, 1], FP32)
    b3_2 = inp("b3_2", [128, 2], FP32)
    wse1_t = inp("wse1_t", [C, 16], BF16)
    wse2_t = inp("wse2_t", [16, C], BF16)
    bse1 = inp("bse1", [16, 1], FP32)
    bse2_2 = inp("bse2_2", [128, 2], FP32)

    out_d = nc.dram_tensor("out", [C, O], FP32, kind="ExternalOutput")
    if dbg:
        dbg_d = {}
        for nm, shape, dt in [
            ("dbg_q", [128, HEADS * O], BF16),
            ("dbg_k", [128, HEADS * 256], BF16),
            ("dbg_vt", [128, 4 * 260], BF16),
            ("dbg_s", [128, 1024], FP32),
            ("dbg_et", [128, 1024], BF16),
            ("dbg_pv", [128, 512], FP32),
            ("dbg_sr", [128, 4 * OC], FP32),
            ("dbg_rr", [128, 4 * OC], FP32),
            ("dbg_rb", [64, OC], FP32),
            ("dbg_x", [128, 2 * OC], BF16),
            ("dbg_h1", [128, OC], BF16),
            ("dbg_sqg", [128, 8], FP32),
            ("dbg_gate", [128, 2], FP32),
        ]:
            dbg_d[nm] = nc.dram_tensor(nm, shape, dt, kind="ExternalOutput")

    with tile.TileContext(nc) as tc:
        with (
            tc.tile_pool(name="const", bufs=1) as cpool,
            tc.tile_pool(name="work", bufs=2) as wpool,
            tc.tile_pool(name="et", bufs=3) as epool,
            tc.tile_pool(name="norm", bufs=1) as npool,
            tc.tile_pool(name="pvc", bufs=8) as pvcpool,
            tc.tile_pool(name="psA", bufs=2, space="PSUM") as psA,
            tc.tile_pool(name="psB", bufs=4, space="PSUM") as psB,
            tc.tile_pool(name="dram", bufs=1, space="DRAM") as dpool,
        ):
            # ---------------- load constants / inputs ----------------
            def load(dram, shape, dt=None, name=None):
                t = cpool.tile(list(shape), dt or dram.dtype, tag=name)
                nc.sync.dma_start(t[:], dram[:])
                return t

            sb_featbf = cpool.tile([128, 2 * N], BF16, tag="featbf")
            for ct in range(2):
                for q4 in range(4):
                    nc.sync.dma_start(
                        sb_featbf[:, ct * N + q4 * 1024: ct * N + (q4 + 1) * 1024],
                        feat_bf[ct * 128:(ct + 1) * 128,
                                q4 * 1024:(q4 + 1) * 1024])
            sb_featq = cpool.tile([128, 2 * O], BF16, tag="featq")
            for ct in range(2):
                nc.sync.dma_start(sb_featq[:, ct * O:(ct + 1) * O],
                                  feat_q[ct * 128:(ct + 1) * 128, :])
            sb_featres = cpool.tile([128, 2 * O], FP32, tag="featres")
            for ct in range(2):
                nc.sync.dma_start(sb_featres[:, ct * O:(ct + 1) * O],
                                  feat_res[ct * 128:(ct + 1) * 128, :])
            sb_featbv4 = cpool.tile([64, HEADS * O], FP32, tag="featbv4")
            nc.sync.dma_start(sb_featbv4[:], feat_bv4[:])

            def load2(dram, cols, name):
                t = cpool.tile([128, 2 * cols], dram.dtype, tag=name)
                for ct in range(2):
                    nc.sync.dma_start(t[:, ct * cols:(ct + 1) * cols],
                                      dram[ct * 128:(ct + 1) * 128, :])
                return t

            sb_wq = load2(wq_t, C, "wq")      # [128, 2*256] lhsT ch-tiles
            sb_wk = load2(wk_t, C, "wk")
            sb_wv = load2(wv_t, C, "wv")
            sb_w1 = load2(w1_t, CH, "w1")     # [128, 2*128]
            sb_w2 = load(w2_t, [128, CH], name="w2")
            sb_w3 = load(w3_t, [128, C], name="w3")
            sb_wse1 = load2(wse1_t, 16, "wse1")
            sb_wse2 = load(wse2_t, [16, C], name="wse2")
            sb_bq2 = load(bq2, [128, 2], name="bq2")
            sb_bk2 = load(bk2, [128, 2], name="bk2")
            sb_bn1s = load(bn1_s, [128, 1], name="bn1s")
            sb_bn1b = load(bn1_b, [128, 1], name="bn1b")
            sb_bn2s = load(bn2_s, [128, 1], name="bn2s")
            sb_bn2b = load(bn2_b, [128, 1], name="bn2b")
            sb_b32 = load(b3_2, [128, 2], name="b32")
            sb_bse1 = load(bse1, [16, 1], name="bse1")
            sb_bse22 = load(bse2_2, [128, 2], name="bse22")

            # ---------------- projections ----------------
            # Q/K psum tiles hold channels [ct*128,(ct+1)*128] = heads 2ct,2ct+1.
            # Cast even head's rows (0:64) and odd head's rows (64:128) straight
            # into the dup tensors (partition-aligned), then DMA-mirror the
            # other half of each.
            q_dup = cpool.tile([128, HEADS * O], BF16, tag="qdup")
            k_dup = cpool.tile([128, HEADS * N], BF16, tag="kdup")
            for ct in range(2):
                he, ho = 2 * ct, 2 * ct + 1
                ps = psA.tile([128, O], FP32, tag="s")
                for ch in range(2):
                    for half in range(2):
                        nc.tensor.matmul(
                            ps[:, half * 512:(half + 1) * 512],
                            sb_wq[:, ch * C + ct * 128: ch * C + (ct + 1) * 128],
                            sb_featq[:, ch * O + half * 512: ch * O + half * 512 + 512],
                            start=(ch == 0), stop=(ch == 1))
                nc.vector.tensor_scalar_add(
                    q_dup[0:64, he * O:(he + 1) * O], ps[0:64, :],
                    sb_bq2[0:64, ct:ct + 1])
                nc.vector.tensor_scalar_add(
                    q_dup[64:128, ho * O:(ho + 1) * O], ps[64:128, :],
                    sb_bq2[64:128, ct:ct + 1])
                for oc4 in range(4):
                    cols = slice(oc4 * 1024, (oc4 + 1) * 1024)
                    psk = psA.tile([128, 1024], FP32, tag="s")
                    for ch in range(2):
                        for half in range(2):
                            nc.tensor.matmul(
                                psk[:, half * 512:(half + 1) * 512],
                                sb_wk[:, ch * C + ct * 128: ch * C + (ct + 1) * 128],
                                sb_featbf[:, ch * N + oc4 * 1024 + half * 512:
                                           ch * N + oc4 * 1024 + half * 512 + 512],
                                start=(ch == 0), stop=(ch == 1))
                    nc.vector.tensor_scalar_add(
                        k_dup[0:64, he * N + oc4 * 1024: he * N + (oc4 + 1) * 1024],
                        psk[0:64, :], sb_bk2[0:64, ct:ct + 1])
                    nc.vector.tensor_scalar_add(
                        k_dup[64:128, ho * N + oc4 * 1024: ho * N + (oc4 + 1) * 1024],
                        psk[64:128, :], sb_bk2[64:128, ct:ct + 1])
            for h in range(4):
                if h % 2 == 0:
                    nc.sync.dma_start(q_dup[64:128, h * O:(h + 1) * O],
                                      q_dup[0:64, h * O:(h + 1) * O])
                    nc.sync.dma_start(k_dup[64:128, h * N:(h + 1) * N],
                                      k_dup[0:64, h * N:(h + 1) * N])
                else:
                    nc.sync.dma_start(q_dup[0:64, h * O:(h + 1) * O],
                                      q_dup[64:128, h * O:(h + 1) * O])
                    nc.sync.dma_start(k_dup[0:64, h * N:(h + 1) * N],
                                      k_dup[64:128, h * N:(h + 1) * N])

            # V^T with ones column: [128, NT * 260]; block (it, h) at
            # cols it*260 + h*65: cols 0-63 = V, col 64 stays 1.0, so the PV
            # matmul (M=65) produces the softmax row sums in psum row 64.
            vt = cpool.tile([128, NT * 260], BF16, tag="vt")
            nc.gpsimd.memset(vt[:], 1.0)
            for it in range(NT):
                ps = psB.tile([128, 256], FP32, tag="pv")
                for ch in range(2):
                    nc.tensor.matmul(
                        ps[:],
                        sb_featbf[:, ch * N + it * 128: ch * N + it * 128 + 128],
                        sb_wv[:, ch * C:(ch + 1) * C],
                        start=(ch == 0), stop=(ch == 1))
                dst = vt[:, it * 260:(it + 1) * 260] \
                    .rearrange("p (h k) -> p h k", k=65)[:, :, 0:64]
                nc.vector.tensor_copy(
                    dst, ps[:].rearrange("p (h k) -> p h k", k=64))

            if dbg:
                nc.sync.dma_start(dbg_d["dbg_q"][:], q_dup[:])
                for h in range(4):
                    nc.sync.dma_start(dbg_d["dbg_k"][:, h * 256:(h + 1) * 256],
                                      k_dup[:, h * N: h * N + 256])
                nc.sync.dma_start(dbg_d["dbg_vt"][:], vt[:, 0:4 * 260])

            # ---------------- attention + conv, per o-chunk ----------------
            msg_sb = cpool.tile([128, 2 * O], FP32, tag="msg")   # conv3 out
            sq_acc = cpool.tile([128, 2], FP32, tag="sqacc")
            nc.vector.memset(sq_acc[:], 0.0)

            for oc in range(O // OC):
                oco = oc * OC
                pv_list = []
                for h in range(4):
                    pv = psB.tile([128, OC], FP32, tag="pv")
                    pv_list.append(pv)
                    for tp in range(NT // 2):
                        i0, i1 = 2 * tp, 2 * tp + 1
                        sps = psA.tile([128, 2 * OC], FP32, tag="s")
                        nc.tensor.matmul(
                            sps[:, 0:OC],
                            k_dup[0:64, h * N + i0 * 128: h * N + (i0 + 1) * 128],
                            q_dup[0:64, h * O + oco: h * O + oco + OC],
                            start=True, stop=True, tile_position=(0, 0))
                        nc.tensor.matmul(
                            sps[:, OC:2 * OC],
                            k_dup[64:128, h * N + i1 * 128: h * N + (i1 + 1) * 128],
                            q_dup[64:128, h * O + oco: h * O + oco + OC],
                            start=True, stop=True, tile_position=(64, 0))
                        et = epool.tile([128, 2 * OC], BF16, tag="et")
                        nc.scalar.activation(et[:], sps[:], ACTF.Exp, scale=0.125)
                        if dbg and oc == 0 and h == 0 and tp == 0:
                            stmp = wpool.tile([128, 1024], FP32, tag="dbgs")
                            nc.vector.tensor_copy(stmp[:], sps[:])
                            nc.sync.dma_start(dbg_d["dbg_s"][:], stmp[:])
                            nc.sync.dma_start(dbg_d["dbg_et"][:], et[:])
                        nc.tensor.matmul(
                            pv[0:65, :],
                            vt[:, i0 * 260 + h * 65: i0 * 260 + h * 65 + 65],
                            et[:, 0:OC],
                            start=(tp == 0), stop=False)
                        nc.tensor.matmul(
                            pv[0:65, :],
                            vt[:, i1 * 260 + h * 65: i1 * 260 + h * 65 + 65],
                            et[:, OC:2 * OC],
                            start=False, stop=(tp == NT // 2 - 1))

                if dbg and oc == 0:
                    pvtmp = wpool.tile([128, 512], FP32, tag="dbgpv")
                    nc.vector.tensor_copy(pvtmp[0:65, :], pv_list[0][0:65, :])
                    nc.sync.dma_start(dbg_d["dbg_pv"][:], pvtmp[:])

                # evict PV psum to SBUF promptly (frees psum for next chunk)
                pvc_list = []
                for h in range(4):
                    pvc = pvcpool.tile([128, OC], FP32, tag="pvc")
                    nc.vector.tensor_copy(pvc[0:65, :], pv_list[h][0:65, :])
                    pvc_list.append(pvc)
                # row sums (row 64 of each head) -> partition-64 row
                s_r = npool.tile([128, 4 * OC], FP32, tag="sr")
                for h in range(4):
                    nc.vector.tensor_copy(s_r[64:65, h * OC:(h + 1) * OC],
                                          pvc_list[h][64:65, :])
                # spread across partitions 64-67 for a cheap exact reciprocal
                s4 = npool.tile([128, OC], FP32, tag="s4")
                nc.sync.dma_start(s4[0:4, :], s_r[64:65, :])
                r4 = npool.tile([128, OC], FP32, tag="r4")
                nc.vector.reciprocal(r4[0:4, :], s4[0:4, :])
                # r row must sit at the tile's partition 0: partition_broadcast
                # ucode broadcasts physical partition 0, not the AP base.
                r_r = npool.tile([128, 4 * OC], FP32, tag="rr")
                nc.sync.dma_start(r_r[0:1, :], r4[0:4, :])

                if dbg and oc == 0:
                    nc.sync.dma_start(dbg_d["dbg_sr"][:], s_r[:])
                    nc.sync.dma_start(dbg_d["dbg_rr"][:], r_r[:])

                # x = feat_bv - M*r  (per head rows), cast bf16
                x_sb = wpool.tile([128, 2 * OC], BF16, tag="x")
                for h in range(4):
                    ct, prow = h // 2, (h % 2) * 64
                    rb = wpool.tile([64, OC], FP32, tag="rb")
                    nc.gpsimd.partition_broadcast(
                        rb[:], r_r[0:1, h * OC:(h + 1) * OC])
                    mr = wpool.tile([64, OC], FP32, tag="mr")
                    nc.vector.tensor_tensor(mr[:], pvc_list[h][0:64, :],
                                            rb[:], ALU.mult)
                    if dbg and oc == 0 and h == 0:
                        nc.sync.dma_start(dbg_d["dbg_rb"][:], rb[:])
                    x_t = wpool.tile([64, OC], BF16, tag="xt")
                    nc.vector.tensor_tensor(
                        x_t[:],
                        sb_featbv4[:, h * O + oco: h * O + oco + OC],
                        mr[:], ALU.subtract)
                    nc.sync.dma_start(
                        x_sb[prow:prow + 64, ct * OC:(ct + 1) * OC], x_t[:])

                if dbg and oc == 0:
                    nc.sync.dma_start(dbg_d["dbg_x"][:], x_sb[:])
                # conv1 -> bn -> relu
                h1 = wpool.tile([128, OC], BF16, tag="h1")
                ps1 = psB.tile([128, OC], FP32, tag="pv")
                for ch in range(2):
                    nc.tensor.matmul(ps1[:], sb_w1[:, ch * CH:(ch + 1) * CH],
                                     x_sb[:, ch * OC:(ch + 1) * OC],
                                     start=(ch == 0), stop=(ch == 1))
                nc.vector.tensor_scalar(h1[:], ps1[:], sb_bn1s[:, 0:1],
                                        sb_bn1b[:, 0:1], ALU.mult, ALU.add)
                nc.vector.tensor_scalar_max(h1[:], h1[:], 0.0)
                if dbg and oc == 0:
                    nc.sync.dma_start(dbg_d["dbg_h1"][:], h1[:])
                # conv2 -> bn -> relu
                h2 = wpool.tile([128, OC], BF16, tag="h2")
                ps2 = psB.tile([128, OC], FP32, tag="pv")
                nc.tensor.matmul(ps2[:], sb_w2[:], h1[:], start=True, stop=True)
                nc.vector.tensor_scalar(h2[:], ps2[:], sb_bn2s[:, 0:1],
                                        sb_bn2b[:, 0:1], ALU.mult, ALU.add)
                nc.vector.tensor_scalar_max(h2[:], h2[:], 0.0)
                # conv3 + b3; accumulate squeeze partial sums
                for ct in range(2):
                    ps3 = psB.tile([128, OC], FP32, tag="pv")
                    nc.tensor.matmul(ps3[:], sb_w3[:, ct * 128:(ct + 1) * 128],
                                     h2[:], start=True, stop=True)
                    sq_c = wpool.tile([128, 1], FP32, tag="sqc")
                    nc.vector.tensor_scalar(
                        msg_sb[:, ct * O + oco: ct * O + oco + OC],
                        ps3[:], sb_b32[:, ct:ct + 1], 0.0, ALU.add, ALU.add,
                        accum_out=sq_c[:])
                    nc.vector.tensor_tensor(sq_acc[:, ct:ct + 1],
                                            sq_acc[:, ct:ct + 1],
                                            sq_c[:], ALU.add)

            # ---------------- SE gate (AllGather over the 4-core group) ------
            cc_in = dpool.tile([128, 2], FP32)
            cc_out = dpool.tile([512, 2], FP32)
            nc.sync.dma_start(cc_in[:], sq_acc[:])
            nc.gpsimd.collective_compute(
                "AllGather", ALU.bypass,
                replica_groups=[[0, 1, 2, 3], [4, 5, 6, 7]],
                ins=[cc_in.opt()], outs=[cc_out.opt()])
            sq_g = wpool.tile([128, 8], FP32, tag="sqg")
            nc.sync.dma_start(
                sq_g[:].rearrange("p (s k) -> p s k", k=2),
                cc_out[:].rearrange("(s p) k -> p s k", p=128))
            if dbg:
                nc.sync.dma_start(dbg_d["dbg_sqg"][:], sq_g[:])
            sq_t = wpool.tile([128, 2], FP32, tag="sqt")
            nc.vector.tensor_tensor(sq_t[:], sq_g[:, 0:2], sq_g[:, 2:4], ALU.add)
            nc.vector.tensor_tensor(sq_t[:], sq_t[:], sq_g[:, 4:6], ALU.add)
            nc.vector.tensor_tensor(sq_t[:], sq_t[:], sq_g[:, 6:8], ALU.add)
            sq_bf = wpool.tile([128, 2], BF16, tag="sqbf")
            nc.vector.tensor_scalar_mul(sq_bf[:], sq_t[:], 1.0 / N)

            fc_ps = psB.tile([128, 2], FP32, tag="pv")
            for ch in range(2):
                nc.tensor.matmul(fc_ps[0:16, 0:1],
                                 sb_wse1[:, ch * 16:(ch + 1) * 16],
                                 sq_bf[:, ch:ch + 1],
                                 start=(ch == 0), stop=(ch == 1))
            fc_sb = wpool.tile([16, 1], BF16, tag="fc")
            nc.vector.tensor_scalar(fc_sb[:], fc_ps[0:16, 0:1], sb_bse1[:, 0:1],
                                    0.0, ALU.add, ALU.max)

            g_ps = psB.tile([128, 2], FP32, tag="pv")
            for ct in range(2):
                nc.tensor.matmul(g_ps[:, ct:ct + 1],
                                 sb_wse2[:, ct * 128:(ct + 1) * 128],
                                 fc_sb[:], start=True, stop=True,
                                 skip_group_check=True)
            # sigmoid(x) = 1/(1+exp(-x)); add bse2 via activation bias
            ge = wpool.tile([128, 2], FP32, tag="ge")
            nc.scalar.activation(ge[:], g_ps[:, 0:2], ACTF.Exp,
                                 bias=sb_bse22[:, 0:1], scale=-1.0)
            # note: bias must be -bse2 with scale=-1: handled on host (bse2_2
            # passed negated), since activation computes func(in*scale + bias).
            nc.vector.tensor_scalar_add(ge[:], ge[:], 1.0)
            gate = wpool.tile([128, 2], FP32, tag="gate")
            nc.vector.reciprocal(gate[:], ge[:])

            if dbg:
                nc.sync.dma_start(dbg_d["dbg_gate"][:], gate[:])
            # out = feat_res + msg * gate  (in place into msg_sb)
            for ct in range(2):
                nc.vector.scalar_tensor_tensor(
                    out=msg_sb[:, ct * O:(ct + 1) * O],
                    in0=msg_sb[:, ct * O:(ct + 1) * O],
                    scalar=gate[:, ct:ct + 1],
                    in1=sb_featres[:, ct * O:(ct + 1) * O],
                    op0=ALU.mult, op1=ALU.add)
                nc.sync.dma_start(out_d[ct * 128:(ct + 1) * 128, :],
                                  msg_sb[:, ct * O:(ct + 1) * O])

    nc.compile()
    return nc


def _prep_inputs(inputs):
    bf = ml_dtypes.bfloat16
    f = lambda x: np.ascontiguousarray(np.asarray(x, dtype=np.float32))
    feat = f(inputs["feat"])
    Wq, Wk, Wv = f(inputs["Wq"]), f(inputs["Wk"]), f(inputs["Wv"])
    bq, bk, bv = f(inputs["bq"]), f(inputs["bk"]), f(inputs["bv"])
    W1, W2, W3 = f(inputs["W1"]), f(inputs["W2"]), f(inputs["W3"])
    b1, b2, b3 = f(inputs["b1"]), f(inputs["b2"]), f(inputs["b3"])
    g1, be1, m1, v1 = f(inputs["g1"]), f(inputs["be1"]), f(inputs["m1"]), f(inputs["v1"])
    g2, be2, m2, v2 = f(inputs["g2"]), f(inputs["be2"]), f(inputs["m2"]), f(inputs["v2"])
    Wse1, Wse2 = f(inputs["Wse1"]), f(inputs["Wse2"])
    bse1, bse2 = f(inputs["bse1"]), f(inputs["bse2"])

    s1 = g1 / np.sqrt(v1 + EPS)
    sh1 = be1 - m1 * s1 + b1 * s1
    s2 = g2 / np.sqrt(v2 + EPS)
    sh2 = be2 - m2 * s2 + b2 * s2

    common = {
        "wq_t": np.ascontiguousarray(Wq.T).astype(bf),
        "wk_t": np.ascontiguousarray(Wk.T).astype(bf),
        "wv_t": np.ascontiguousarray(Wv.T).astype(bf),
        "bq2": np.ascontiguousarray(bq.reshape(2, 128).T),
        "bk2": np.ascontiguousarray(bk.reshape(2, 128).T),
        "w1_t": np.ascontiguousarray(W1.T).astype(bf),
        "w2_t": np.ascontiguousarray(W2.T).astype(bf),
        "w3_t": np.ascontiguousarray(W3.T).astype(bf),
        "bn1_s": s1.reshape(128, 1),
        "bn1_b": sh1.reshape(128, 1),
        "bn2_s": s2.reshape(128, 1),
        "bn2_b": sh2.reshape(128, 1),
        "b3_2": np.ascontiguousarray(b3.reshape(2, 128).T),
        "wse1_t": np.ascontiguousarray(Wse1.T).astype(bf),
        "wse2_t": np.ascontiguousarray(Wse2.T).astype(bf),
        "bse1": bse1.reshape(16, 1),
        "bse2_2": np.ascontiguousarray((-bse2).reshape(2, 128).T),
    }

    in_maps = []
    for core in range(8):
        b, osl = core // 4, core % 4
        o0 = osl * O
        fb = feat[b]
        m = dict(common)
        m["feat_bf"] = fb.astype(bf)
        m["feat_q"] = np.ascontiguousarray(fb[:, o0:o0 + O]).astype(bf)
        m["feat_res"] = np.ascontiguousarray(fb[:, o0:o0 + O])
        fbv = fb[:, o0:o0 + O] - bv[:, None]
        m["feat_bv4"] = np.ascontiguousarray(
            np.concatenate([fbv[64 * h:64 * h + 64, :] for h in range(4)], axis=1))
        in_maps.append(m)
    return in_maps


def kernel(**inputs) -> np.ndarray:
    if "nc" not in _CACHE:
        _CACHE["nc"] = _build()
    nc = _CACHE["nc"]
    in_maps = _prep_inputs(inputs)
    res = run_bass_kernel_spmd(nc, in_maps, core_ids=list(range(8)))
    out = np.zeros((BS, C, N), dtype=np.float32)
    for core in range(8):
        b, osl = core // 4, core % 4
        out[b, :, osl * O:(osl + 1) * O] = res.results[core]["out"]
    return out


if __name__ == "__main__":
    import sys
    sys.path.insert(0, "/root/problem")
    from reference import setup_inputs, reference
    inp = {k: np.asarray(v) for k, v in setup_inputs().items()}
    ref = np.asarray(reference(**inp))
    got = kernel(**inp)
    err = np.abs(got - ref)
    print("absmax err:", err.max(), "ref absmax:", np.abs(ref).max())
    print("Relative error:", err.max() / np.abs(ref).max())



# revision 5
# speedup vs baseline: 1.1033x; 1.1033x over previous
"""Trainium2 Bass kernel for nn_NonLocalBlock (multi-head non-local attention
block with conv/BN/SE tail).

Sharding: 8 cores = 2 batches x 4 query(o)-slices of 1024. Each core computes
full attention (all 4 heads, full key length 4096) for its o-slice, the conv
stack on its slice, and joins the SE squeeze via per-chunk 4-core AllGathers.

Key structure:
 - Head-pair layout: Q/K live as [128, cols] tiles where rows 0:64 = even
   head dims, 64:128 = odd head dims (the natural projection-psum layout).
   QK quad-packs the two heads at tile_position (0,0)/(64,0) - no dup DMAs.
 - K bias dropped: softmax over keys is invariant to per-query constants,
   so (Q+bq)*(K+bk) == (Q+bq)*K modulo softmax.
 - exp split across engines: 5/8 of score tiles on ScalarE (native Exp),
   3/8 on DVE via Schraudolph bit-trick: int16(round(s*23.083+16250.5))
   bitcast to bf16 ~= exp(s/8) within 3.3%; softmax cancels most of it.
 - softmax normalized AFTER PV: ones-column appended to V^T (M=65) yields
   row sums in psum row 64 of the same matmul.
 - BN folded into weights (host); BN-shift + relu on ScalarE activation;
   h2 activation's accum_out produces the SE squeeze partial for free
   (sq path folded on host: fc = relu((Wse1@W3/N) @ sum(h2) + Wse1@b3+bse1)).
"""
import numpy as np
import ml_dtypes

import concourse.bass as bass
import concourse.tile as tile
from concourse import bacc, mybir
from concourse.bass_utils import run_bass_kernel_spmd

FP32 = mybir.dt.float32
BF16 = mybir.dt.bfloat16
I16 = mybir.dt.int16
ALU = mybir.AluOpType
ACTF = mybir.ActivationFunctionType

C, CH, N, BS, HEADS, DH = 256, 128, 4096, 2, 4, 64
O = 1024          # per-core o-slice
OC = 512          # o-chunk
NT = N // 128     # 32 i-tiles
EPS = 1e-5

# Schraudolph bf16 exp-from-bits: bits = round(x*0.125 * 128*log2(e) + B)
SCH_A = 0.125 * 128.0 * np.log2(np.e)
SCH_B = 127.0 * 128.0 - 5.5

_CACHE = {}


def _build():
    nc = bacc.Bacc(None, target_bir_lowering=False, debug=False)

    di = {}
    def inp(name, shape, dt):
        di[name] = nc.dram_tensor(name, list(shape), dt, kind="ExternalInput")
        return di[name]

    feat_bf = inp("feat_bf", [C, N], BF16)
    feat_q = inp("feat_q", [C, O], BF16)
    feat_res = inp("feat_res", [C, O], FP32)
    feat_bv4 = inp("feat_bv4", [64, HEADS * O], BF16)
    wq_t = inp("wq_t", [C, C], BF16)
    wk_t = inp("wk_t", [C, C], BF16)
    wv_t = inp("wv_t", [C, C], BF16)
    bq2 = inp("bq2", [128, 2], FP32)
    w1_4 = inp("w1_4", [64, 4 * CH], BF16)     # conv1 lhsT, 4 head-chunks
    w2_t = inp("w2_t", [CH, CH], BF16)
    w3_t = inp("w3_t", [CH, C], BF16)
    bn1_b = inp("bn1_b", [128, 1], FP32)
    bn2_b = inp("bn2_b", [128, 1], FP32)
    b3_2 = inp("b3_2", [128, 2], FP32)
    wsq_t = inp("wsq_t", [CH, 16], BF16)       # (Wse1 @ W3 / N)^T
    bsq = inp("bsq", [16, 1], FP32)            # Wse1 @ b3 + bse1
    wse2_t = inp("wse2_t", [16, C], BF16)
    bse2_2 = inp("bse2_2", [128, 2], FP32)     # negated

    out_d = nc.dram_tensor("out", [C, O], FP32, kind="ExternalOutput")

    with tile.TileContext(nc) as tc:
        with (
            tc.tile_pool(name="const", bufs=1) as cpool,
            tc.tile_pool(name="work", bufs=2) as wpool,
            tc.tile_pool(name="et", bufs=3) as epool,
            tc.tile_pool(name="norm", bufs=2) as npool,
            tc.tile_pool(name="psA", bufs=2, space="PSUM") as psA,
            tc.tile_pool(name="psB", bufs=2, space="PSUM") as psB,
            tc.tile_pool(name="dram", bufs=1, space="DRAM") as dpool,
        ):
            # ---------------- load constants / inputs ----------------
            def load(dram, shape, name):
                t = cpool.tile(list(shape), dram.dtype, tag=name)
                nc.sync.dma_start(t[:], dram[:])
                return t

            sb_featbf = cpool.tile([128, 2 * N], BF16, tag="featbf")
            for ct in range(2):
                for q4 in range(4):
                    nc.sync.dma_start(
                        sb_featbf[:, ct * N + q4 * 1024: ct * N + (q4 + 1) * 1024],
                        feat_bf[ct * 128:(ct + 1) * 128,
                                q4 * 1024:(q4 + 1) * 1024])
            sb_featres = cpool.tile([128, 2 * O], FP32, tag="featres")
            for ct in range(2):
                nc.sync.dma_start(sb_featres[:, ct * O:(ct + 1) * O],
                                  feat_res[ct * 128:(ct + 1) * 128, :])
            sb_featbv4 = load(feat_bv4, [64, HEADS * O], "featbv4")

            def load2(dram, cols, name):
                t = cpool.tile([128, 2 * cols], dram.dtype, tag=name)
                for ct in range(2):
                    nc.sync.dma_start(t[:, ct * cols:(ct + 1) * cols],
                                      dram[ct * 128:(ct + 1) * 128, :])
                return t

            sb_featq = load2(feat_q, O, "featq")
            sb_wq = load2(wq_t, C, "wq")      # [128, 2*256] lhsT ch-tiles
            sb_wk = load2(wk_t, C, "wk")
            sb_wv = load2(wv_t, C, "wv")
            sb_w14 = load(w1_4, [64, 4 * CH], "w14")
            sb_w2 = load(w2_t, [128, CH], "w2")
            sb_w3 = load(w3_t, [128, C], "w3")
            sb_wsq = load(wsq_t, [128, 16], "wsq")
            sb_wse2 = load(wse2_t, [16, C], "wse2")
            sb_bq2 = load(bq2, [128, 2], "bq2")
            sb_bn1b = load(bn1_b, [128, 1], "bn1b")
            sb_bn2b = load(bn2_b, [128, 1], "bn2b")
            sb_b32 = load(b3_2, [128, 2], "b32")
            sb_bsq = load(bsq, [16, 1], "bsq")
            sb_bse22 = load(bse2_2, [128, 2], "bse22")

            # ---------------- V^T with ones column ----------------
            # [128, NT*260]; block (it, h) at cols it*260 + h*65: cols 0-63
            # = V rows, col 64 stays 1.0 -> PV matmul (M=65) emits row sums.
            vt = cpool.tile([128, NT * 260], BF16, tag="vt")
            nc.gpsimd.memset(vt[:], 1.0)
            for itp in range(NT // 2):
                ps = psB.tile([128, 2 * OC], FP32, tag="pv")
                for half in range(2):
                    it = 2 * itp + half
                    for ch in range(2):
                        nc.tensor.matmul(
                            ps[:, half * 256:half * 256 + 256],
                            sb_featbf[:, ch * N + it * 128: ch * N + it * 128 + 128],
                            sb_wv[:, ch * C:(ch + 1) * C],
                            start=(ch == 0), stop=(ch == 1),
                            skip_group_check=True)
                dst = vt[:, itp * 520:(itp + 1) * 520] \
                    .rearrange("p (i h k) -> p i h k", i=2, k=65)[:, :, :, 0:64]
                nc.vector.tensor_copy(
                    dst, ps[:, 0:512].rearrange("p (i h k) -> p i h k",
                                                i=2, k=64))

            # ---------------- K / Q projections (head-pair layout) --------
            # kp ct-block: [128, N] rows 0:64 = head 2ct dims, 64:128 = 2ct+1
            kp = cpool.tile([128, 2 * N], BF16, tag="kp")
            qp = cpool.tile([128, 2 * O], BF16, tag="qp")
            for ct in range(2):
                for oc4 in range(4):
                    psk = psA.tile([128, 2 * OC], FP32, tag="s")
                    for half in range(2):
                        for ch in range(2):
                            nc.tensor.matmul(
                                psk[:, half * OC:(half + 1) * OC],
                                sb_wk[:, ch * C + ct * 128: ch * C + (ct + 1) * 128],
                                sb_featbf[:, ch * N + oc4 * 1024 + half * OC:
                                          ch * N + oc4 * 1024 + (half + 1) * OC],
                                start=(ch == 0), stop=(ch == 1))
                    nc.vector.tensor_copy(
                        kp[:, ct * N + oc4 * 1024: ct * N + (oc4 + 1) * 1024],
                        psk[:])
                psq = psA.tile([128, 2 * OC], FP32, tag="s")
                for half in range(2):
                    for ch in range(2):
                        nc.tensor.matmul(
                            psq[:, half * OC:(half + 1) * OC],
                            sb_wq[:, ch * C + ct * 128: ch * C + (ct + 1) * 128],
                            sb_featq[:, ch * O + half * OC:
                                     ch * O + (half + 1) * OC],
                            start=(ch == 0), stop=(ch == 1))
                nc.vector.tensor_scalar_add(
                    qp[:, ct * O:(ct + 1) * O], psq[:], sb_bq2[:, ct:ct + 1])

            # ---------------- attention ----------------
            msg_sb = cpool.tile([128, 2 * O], FP32, tag="msg")
            pvs = {}      # (oc, ct) -> pv psum tile
            x4s = {}      # oc -> conv1 input [64, 4*OC]
            cc_outs = {}

            def attn_pair(oc, ct, it_lo, it_hi):
                oco = oc * OC
                if (oc, ct) not in pvs:
                    pvs[(oc, ct)] = psB.tile([128, 2 * OC], FP32, tag="pv",
                                             name=f"pv{oc}{ct}")
                pvp = pvs[(oc, ct)]
                for it in range(it_lo, it_hi):
                    sps = psA.tile([128, 2 * OC], FP32, tag="s")
                    nc.tensor.matmul(
                        sps[:, 0:OC],
                        kp[0:64, ct * N + it * 128: ct * N + (it + 1) * 128],
                        qp[0:64, ct * O + oco: ct * O + oco + OC],
                        start=True, stop=True, tile_position=(0, 0))
                    nc.tensor.matmul(
                        sps[:, OC:2 * OC],
                        kp[64:128, ct * N + it * 128: ct * N + (it + 1) * 128],
                        qp[64:128, ct * O + oco: ct * O + oco + OC],
                        start=True, stop=True, tile_position=(64, 0))
                    if it % 8 < 5:
                        et = epool.tile([128, 2 * OC], BF16, tag="et")
                        nc.scalar.activation(et[:], sps[:], ACTF.Exp,
                                             scale=0.125)
                        el, er = et[:, 0:OC], et[:, OC:2 * OC]
                    else:
                        eti = epool.tile([128, 2 * OC], I16, tag="eti")
                        nc.vector.tensor_scalar(
                            eti[:], sps[:], SCH_A, SCH_B, ALU.mult, ALU.add)
                        el = eti[:, 0:OC].bitcast(BF16)
                        er = eti[:, OC:2 * OC].bitcast(BF16)
                    nc.tensor.matmul(
                        pvp[0:65, 0:OC],
                        vt[:, it * 260 + 2 * ct * 65:
                           it * 260 + 2 * ct * 65 + 65],
                        el, start=(it == 0), stop=(it == NT - 1))
                    nc.tensor.matmul(
                        pvp[0:65, OC:2 * OC],
                        vt[:, it * 260 + (2 * ct + 1) * 65:
                           it * 260 + (2 * ct + 1) * 65 + 65],
                        er, start=(it == 0), stop=(it == NT - 1))

            def norm_pair_s(oc, ct, srow):
                # row sums (psum row 64) -> srow partition 64, head-major cols
                nc.vector.tensor_copy(
                    srow[64:65, ct * 2 * OC:(ct + 1) * 2 * OC],
                    pvs[(oc, ct)][64:65, :])

            def norm_chunk(oc, srow):
                # sums -> partitions 0-3, reciprocal, back to partition 0
                s4 = npool.tile([4, OC], FP32, tag="s4")
                nc.sync.dma_start(s4[:], srow[64:65, :])
                r4 = npool.tile([4, OC], FP32, tag="r4")
                nc.vector.reciprocal(r4[:], s4[:])
                rflat = npool.tile([1, 4 * OC], FP32, tag="rflat")
                nc.sync.dma_start(rflat[:], r4[:])
                x4 = wpool.tile([64, 4 * OC], BF16, tag="x4")
                x4s[oc] = x4
                for ct in range(2):
                    rb = npool.tile([64, 2 * OC], FP32, tag="rb")
                    nc.gpsimd.partition_broadcast(
                        rb[:], rflat[0:1, ct * 2 * OC:(ct + 1) * 2 * OC])
                    mr = wpool.tile([64, 2 * OC], BF16, tag="mr")
                    nc.vector.tensor_tensor(
                        mr[:], pvs[(oc, ct)][0:64, :], rb[:], ALU.mult)
                    nc.vector.tensor_tensor(
                        x4[:, ct * 2 * OC:(ct + 1) * 2 * OC]
                        .rearrange("p (h q) -> p h q", h=2),
                        sb_featbv4[:, 2 * ct * O:(2 * ct + 2) * O]
                        .rearrange("p (h o) -> p h o", h=2)[
                            :, :, oc * OC:oc * OC + OC],
                        mr[:].rearrange("p (h q) -> p h q", h=2),
                        ALU.subtract)

            def conv_chunk(oc):
                oco = oc * OC
                x4 = x4s[oc]
                ps1 = psA.tile([128, 2 * OC], FP32, tag="s")
                for h in range(4):
                    nc.tensor.matmul(
                        ps1[:, 0:OC], sb_w14[:, h * CH:(h + 1) * CH],
                        x4[:, h * OC:(h + 1) * OC],
                        start=(h == 0), stop=(h == 3))
                h1 = wpool.tile([128, OC], BF16, tag="h1")
                nc.scalar.activation(h1[:], ps1[:, 0:OC], ACTF.Relu,
                                     bias=sb_bn1b[:, 0:1])
                ps2 = psA.tile([128, 2 * OC], FP32, tag="s")
                nc.tensor.matmul(ps2[:, 0:OC], sb_w2[:], h1[:],
                                 start=True, stop=True)
                h2 = wpool.tile([128, OC], BF16, tag="h2")
                h2s = cpool.tile([128, 1], FP32, tag=f"h2s{oc}")
                nc.scalar.activation(h2[:], ps2[:, 0:OC], ACTF.Relu,
                                     bias=sb_bn2b[:, 0:1], accum_out=h2s[:])
                ps3 = psA.tile([128, 2 * OC], FP32, tag="s")
                for ct in range(2):
                    nc.tensor.matmul(
                        ps3[:, ct * OC:(ct + 1) * OC],
                        sb_w3[:, ct * 128:(ct + 1) * 128], h2[:],
                        start=True, stop=True, skip_group_check=True)
                for ct in range(2):
                    nc.vector.tensor_scalar_add(
                        msg_sb[:, ct * O + oco: ct * O + oco + OC],
                        ps3[:, ct * OC:(ct + 1) * OC], sb_b32[:, ct:ct + 1])
                # SE squeeze partial: AllGather h2 column-sums across 4 cores
                cc_in = dpool.tile([128, 1], FP32, tag=f"cci{oc}")
                cc_out = dpool.tile([512, 1], FP32, tag=f"cco{oc}")
                cc_outs[oc] = cc_out
                nc.sync.dma_start(cc_in[:], h2s[:])
                nc.gpsimd.collective_compute(
                    "AllGather", ALU.bypass,
                    replica_groups=[[0, 1, 2, 3], [4, 5, 6, 7]],
                    ins=[cc_in.opt()], outs=[cc_out.opt()])

            # emission order: attn(0) | attn(1) partial | conv(0)+cc |
            # attn(1) rest | conv(1)+cc | tail.  Keeps the PE queue free of
            # stalls while the first collective overlaps chunk-1 attention.
            srow0 = npool.tile([128, 4 * OC], FP32, tag="sr")
            srow1 = npool.tile([128, 4 * OC], FP32, tag="sr")
            attn_pair(0, 0, 0, NT)
            norm_pair_s(0, 0, srow0)
            attn_pair(0, 1, 0, NT)
            norm_pair_s(0, 1, srow0)
            norm_chunk(0, srow0)
            attn_pair(1, 0, 0, 12)
            conv_chunk(0)
            attn_pair(1, 0, 12, NT)
            norm_pair_s(1, 0, srow1)
            attn_pair(1, 1, 0, NT)
            norm_pair_s(1, 1, srow1)
            norm_chunk(1, srow1)
            conv_chunk(1)

            # ---------------- SE gate tail ----------------
            sq_g = wpool.tile([128, 8], FP32, tag="sqg")
            for oc in range(2):
                nc.sync.dma_start(
                    sq_g[:, 4 * oc:4 * oc + 4],
                    cc_outs[oc][:].rearrange("(s p) k -> p (s k)", p=128))
            sq_t = wpool.tile([128, 4], FP32, tag="sqt")
            nc.vector.tensor_tensor(sq_t[:], sq_g[:, 0:4], sq_g[:, 4:8],
                                    ALU.add)
            sq_t2 = wpool.tile([128, 2], FP32, tag="sqt2")
            nc.vector.tensor_tensor(sq_t2[:], sq_t[:, 0:2], sq_t[:, 2:4],
                                    ALU.add)
            hs_bf = wpool.tile([128, 1], BF16, tag="hsbf")
            nc.vector.tensor_tensor(hs_bf[:], sq_t2[:, 0:1], sq_t2[:, 1:2],
                                    ALU.add)

            fc_ps = psA.tile([128, 2 * OC], FP32, tag="s")
            nc.tensor.matmul(fc_ps[0:16, 0:1], sb_wsq[:, 0:16], hs_bf[:],
                             start=True, stop=True)
            fc_sb = wpool.tile([16, 1], BF16, tag="fc")
            nc.vector.tensor_scalar(fc_sb[:], fc_ps[0:16, 0:1],
                                    sb_bsq[:, 0:1], 0.0, ALU.add, ALU.max)

            g_ps = psB.tile([128, 2 * OC], FP32, tag="pv")
            for ct in range(2):
                nc.tensor.matmul(g_ps[:, ct:ct + 1],
                                 sb_wse2[:, ct * 128:(ct + 1) * 128],
                                 fc_sb[:], start=True, stop=True,
                                 skip_group_check=True)
            # sigmoid(x) = 1/(1+exp(-x)); bse2 negated on host
            ge = wpool.tile([128, 2], FP32, tag="ge")
            for ct in range(2):
                nc.scalar.activation(ge[:, ct:ct + 1], g_ps[:, ct:ct + 1],
                                     ACTF.Exp, bias=sb_bse22[:, ct:ct + 1],
                                     scale=-1.0)
            nc.vector.tensor_scalar_add(ge[:], ge[:], 1.0)
            gate = wpool.tile([128, 2], FP32, tag="gate")
            nc.vector.reciprocal(gate[:], ge[:])

            # out = feat_res + msg * gate
            for ct in range(2):
                nc.vector.scalar_tensor_tensor(
                    out=msg_sb[:, ct * O:(ct + 1) * O],
                    in0=msg_sb[:, ct * O:(ct + 1) * O],
                    scalar=gate[:, ct:ct + 1],
                    in1=sb_featres[:, ct * O:(ct + 1) * O],
                    op0=ALU.mult, op1=ALU.add)
                nc.sync.dma_start(out_d[ct * 128:(ct + 1) * 128, :],
                                  msg_sb[:, ct * O:(ct + 1) * O])

    nc.compile()
    return nc


def _prep_inputs(inputs):
    bf = ml_dtypes.bfloat16
    f = lambda x: np.ascontiguousarray(np.asarray(x, dtype=np.float32))
    feat = f(inputs["feat"])
    Wq, Wk, Wv = f(inputs["Wq"]), f(inputs["Wk"]), f(inputs["Wv"])
    bq, bv = f(inputs["bq"]), f(inputs["bv"])
    W1, W2, W3 = f(inputs["W1"]), f(inputs["W2"]), f(inputs["W3"])
    b1, b2, b3 = f(inputs["b1"]), f(inputs["b2"]), f(inputs["b3"])
    g1, be1, m1, v1 = f(inputs["g1"]), f(inputs["be1"]), f(inputs["m1"]), f(inputs["v1"])
    g2, be2, m2, v2 = f(inputs["g2"]), f(inputs["be2"]), f(inputs["m2"]), f(inputs["v2"])
    Wse1, Wse2 = f(inputs["Wse1"]), f(inputs["Wse2"])
    bse1, bse2 = f(inputs["bse1"]), f(inputs["bse2"])

    s1 = g1 / np.sqrt(v1 + EPS)
    sh1 = be1 - m1 * s1 + b1 * s1
    W1p = W1 * s1[:, None]
    s2 = g2 / np.sqrt(v2 + EPS)
    sh2 = be2 - m2 * s2 + b2 * s2
    W2p = W2 * s2[:, None]

    w1_4 = np.concatenate(
        [np.ascontiguousarray(W1p[:, 64 * h:64 * h + 64].T) for h in range(4)],
        axis=1)                                            # [64, 4*128]
    wsq = (Wse1 @ W3) / np.float32(N)                      # [16, 128]
    bsqv = Wse1 @ b3 + bse1                                # [16]

    common = {
        "wq_t": np.ascontiguousarray(Wq.T).astype(bf),
        "wk_t": np.ascontiguousarray(Wk.T).astype(bf),
        "wv_t": np.ascontiguousarray(Wv.T).astype(bf),
        "bq2": np.ascontiguousarray(bq.reshape(2, 128).T),
        "w1_4": np.ascontiguousarray(w1_4).astype(bf),
        "w2_t": np.ascontiguousarray(W2p.T).astype(bf),
        "w3_t": np.ascontiguousarray(W3.T).astype(bf),
        "bn1_b": sh1.reshape(128, 1),
        "bn2_b": sh2.reshape(128, 1),
        "b3_2": np.ascontiguousarray(b3.reshape(2, 128).T),
        "wsq_t": np.ascontiguousarray(wsq.T).astype(bf),
        "bsq": bsqv.reshape(16, 1),
        "wse2_t": np.ascontiguousarray(Wse2.T).astype(bf),
        "bse2_2": np.ascontiguousarray((-bse2).reshape(2, 128).T),
    }

    in_maps = []
    for core in range(8):
        b, osl = core // 4, core % 4
        o0 = osl * O
        fb = feat[b]
        m = dict(common)
        m["feat_bf"] = fb.astype(bf)
        m["feat_q"] = np.ascontiguousarray(fb[:, o0:o0 + O]).astype(bf)
        m["feat_res"] = np.ascontiguousarray(fb[:, o0:o0 + O])
        fbv = fb[:, o0:o0 + O] - bv[:, None]
        m["feat_bv4"] = np.ascontiguousarray(
            np.concatenate([fbv[64 * h:64 * h + 64, :] for h in range(4)],
                           axis=1)).astype(bf)
        in_maps.append(m)
    return in_maps


def kernel(**inputs) -> np.ndarray:
    if "nc" not in _CACHE:
        _CACHE["nc"] = _build()
    nc = _CACHE["nc"]
    in_maps = _prep_inputs(inputs)
    res = run_bass_kernel_spmd(nc, in_maps, core_ids=list(range(8)))
    out = np.zeros((BS, C, N), dtype=np.float32)
    for core in range(8):
        b, osl = core // 4, core % 4
        out[b, :, osl * O:(osl + 1) * O] = res.results[core]["out"]
    return out


if __name__ == "__main__":
    import sys
    sys.path.insert(0, "/root/problem")
    from reference import setup_inputs, reference
    inp = {k: np.asarray(v) for k, v in setup_inputs().items()}
    ref = np.asarray(reference(**inp))
    got = kernel(**inp)
    err = np.abs(got - ref)
    print("absmax err:", err.max(), "ref absmax:", np.abs(ref).max())
    print("Relative error:", err.max() / np.abs(ref).max())


# revision 7
# speedup vs baseline: 1.1195x; 1.0147x over previous
"""Trainium2 Bass kernel for nn_NonLocalBlock (multi-head non-local attention
block with conv/BN/SE tail).

Sharding: 8 cores = 2 batches x 4 query(o)-slices of 1024. Each core computes
full attention (all 4 heads, full key length 4096) for its o-slice, the conv
stack on its slice, and joins the SE squeeze via per-chunk 4-core AllGathers.

Key structure:
 - Head-pair layout: Q/K live as [128, cols] tiles where rows 0:64 = even
   head dims, 64:128 = odd head dims (the natural projection-psum layout).
   QK quad-packs the two heads at tile_position (0,0)/(64,0) - no dup DMAs.
 - K bias dropped: softmax over keys is invariant to per-query constants,
   so (Q+bq)*(K+bk) == (Q+bq)*K modulo softmax.
 - exp split across engines: 5/8 of score tiles on ScalarE (native Exp),
   3/8 on DVE via Schraudolph bit-trick: int16(round(s*23.083+16250.5))
   bitcast to bf16 ~= exp(s/8) within 3.3%; softmax cancels most of it.
 - The attention stream is software-pipelined one unit ahead (QK of unit
   k+1 emitted before exp/PV of unit k) so the strict-FIFO PE queue never
   blocks on the exp engines.
 - softmax normalized AFTER PV: ones-column appended to V^T (M=65) yields
   row sums in psum row 64 of the same matmul.
 - BN folded into weights (host); BN-shift + relu on ScalarE activation;
   h2 activation's accum_out produces the SE squeeze partial for free
   (sq path folded on host: fc = relu((Wse1@W3/N) @ sum(h2) + Wse1@b3+bse1)).
"""
import numpy as np
import ml_dtypes

import concourse.bass as bass
import concourse.tile as tile
from concourse import bacc, mybir
from concourse.bass_utils import run_bass_kernel_spmd

FP32 = mybir.dt.float32
BF16 = mybir.dt.bfloat16
I16 = mybir.dt.int16
ALU = mybir.AluOpType
ACTF = mybir.ActivationFunctionType

C, CH, N, BS, HEADS, DH = 256, 128, 4096, 2, 4, 64
O = 1024          # per-core o-slice
OC = 512          # o-chunk
NT = N // 128     # 32 i-tiles
EPS = 1e-5

# Schraudolph bf16 exp-from-bits: bits = round(x*0.125 * 128*log2(e) + B)
SCH_A = 0.125 * 128.0 * np.log2(np.e)
SCH_B = 127.0 * 128.0 - 5.5
SC_PAT = {0, 2, 4, 5, 7}   # it%8 values handled by ScalarE (5/8)

_CACHE = {}


def _build():
    nc = bacc.Bacc(None, target_bir_lowering=False, debug=False)

    di = {}
    def inp(name, shape, dt):
        di[name] = nc.dram_tensor(name, list(shape), dt, kind="ExternalInput")
        return di[name]

    feat_bf = inp("feat_bf", [C, N], BF16)
    feat_q = inp("feat_q", [C, O], BF16)
    feat_res = inp("feat_res", [C, O], FP32)
    feat_bv4 = inp("feat_bv4", [64, HEADS * O], BF16)
    wq_t = inp("wq_t", [C, C], BF16)
    wk_t = inp("wk_t", [C, C], BF16)
    wv_t = inp("wv_t", [C, C], BF16)
    bq2 = inp("bq2", [128, 2], FP32)
    w1_4 = inp("w1_4", [64, 4 * CH], BF16)     # conv1 lhsT, 4 head-chunks
    w2_t = inp("w2_t", [CH, CH], BF16)
    w3_t = inp("w3_t", [CH, C], BF16)
    bn1_b = inp("bn1_b", [128, 1], FP32)
    bn2_b = inp("bn2_b", [128, 1], FP32)
    b3_2 = inp("b3_2", [128, 2], FP32)
    wsq_t = inp("wsq_t", [CH, 16], BF16)       # (Wse1 @ W3 / N)^T
    bsq = inp("bsq", [16, 1], FP32)            # Wse1 @ b3 + bse1
    wse2_t = inp("wse2_t", [16, C], BF16)
    bse2_2 = inp("bse2_2", [128, 2], FP32)     # negated

    out_d = nc.dram_tensor("out", [C, O], FP32, kind="ExternalOutput")

    with tile.TileContext(nc) as tc:
        with (
            tc.tile_pool(name="const", bufs=1) as cpool,
            tc.tile_pool(name="work", bufs=2) as wpool,
            tc.tile_pool(name="et", bufs=3) as epool,
            tc.tile_pool(name="norm", bufs=2) as npool,
            tc.tile_pool(name="psA", bufs=2, space="PSUM") as psA,
            tc.tile_pool(name="psB", bufs=2, space="PSUM") as psB,
            tc.tile_pool(name="dram", bufs=1, space="DRAM") as dpool,
        ):
            # ---------------- load constants / inputs ----------------
            def load(dram, shape, name):
                t = cpool.tile(list(shape), dram.dtype, tag=name, name=name)
                nc.sync.dma_start(t[:], dram[:])
                return t

            sb_featbf = cpool.tile([128, 2 * N], BF16, tag="featbf")
            for ct in range(2):
                for q4 in range(4):
                    nc.sync.dma_start(
                        sb_featbf[:, ct * N + q4 * 1024: ct * N + (q4 + 1) * 1024],
                        feat_bf[ct * 128:(ct + 1) * 128,
                                q4 * 1024:(q4 + 1) * 1024])
            sb_featres = cpool.tile([128, 2 * O], FP32, tag="featres")
            for ct in range(2):
                nc.sync.dma_start(sb_featres[:, ct * O:(ct + 1) * O],
                                  feat_res[ct * 128:(ct + 1) * 128, :])
            sb_featbv4 = load(feat_bv4, [64, HEADS * O], "featbv4")

            def load2(dram, cols, name):
                t = cpool.tile([128, 2 * cols], dram.dtype, tag=name, name=name)
                for ct in range(2):
                    nc.sync.dma_start(t[:, ct * cols:(ct + 1) * cols],
                                      dram[ct * 128:(ct + 1) * 128, :])
                return t

            sb_featq = load2(feat_q, O, "featq")
            sb_wq = load2(wq_t, C, "wq")      # [128, 2*256] lhsT ch-tiles
            sb_wk = load2(wk_t, C, "wk")
            sb_wv = load2(wv_t, C, "wv")
            sb_w14 = load(w1_4, [64, 4 * CH], "w14")
            sb_w2 = load(w2_t, [128, CH], "w2")
            sb_w3 = load(w3_t, [128, C], "w3")
            sb_wsq = load(wsq_t, [128, 16], "wsq")
            sb_wse2 = load(wse2_t, [16, C], "wse2")
            sb_bq2 = load(bq2, [128, 2], "bq2")
            sb_bn1b = load(bn1_b, [128, 1], "bn1b")
            sb_bn2b = load(bn2_b, [128, 1], "bn2b")
            sb_b32 = load(b3_2, [128, 2], "b32")
            sb_bsq = load(bsq, [16, 1], "bsq")
            sb_bse22 = load(bse2_2, [128, 2], "bse22")

            # ---------------- V^T with ones column ----------------
            # [128, NT*260]; block (it, h) at cols it*260 + h*65: cols 0-63
            # = V rows, col 64 stays 1.0 -> PV matmul (M=65) emits row sums.
            # PSUM->SBUF evictions alternate DVE / ScalarE to balance load.
            vt = cpool.tile([128, NT * 260], BF16, tag="vt")
            nc.gpsimd.memset(vt[:], 1.0)
            for itp in range(NT // 2):
                ps = psB.tile([128, 2 * OC], FP32, tag="pv", name="vps")
                for half in range(2):
                    it = 2 * itp + half
                    for ch in range(2):
                        nc.tensor.matmul(
                            ps[:, half * 256:half * 256 + 256],
                            sb_featbf[:, ch * N + it * 128: ch * N + it * 128 + 128],
                            sb_wv[:, ch * C:(ch + 1) * C],
                            start=(ch == 0), stop=(ch == 1),
                            skip_group_check=True)
                dst = vt[:, itp * 520:(itp + 1) * 520] \
                    .rearrange("p (i h k) -> p i h k", i=2, k=65)[:, :, :, 0:64]
                src = ps[:, 0:512].rearrange("p (i h k) -> p i h k", i=2, k=64)
                if itp % 2 == 0:
                    nc.vector.tensor_copy(dst, src)
                else:
                    nc.scalar.activation(dst, src, ACTF.Copy)

            # ---------------- K / Q projections (head-pair layout) --------
            # kp ct-block: [128, N] rows 0:64 = head 2ct dims, 64:128 = 2ct+1
            kp = cpool.tile([128, 2 * N], BF16, tag="kp")
            qp = cpool.tile([128, 2 * O], BF16, tag="qp")
            for ct in range(2):
                for oc4 in range(4):
                    psk = psA.tile([128, 2 * OC], FP32, tag="s", name="psk")
                    for half in range(2):
                        for ch in range(2):
                            nc.tensor.matmul(
                                psk[:, half * OC:(half + 1) * OC],
                                sb_wk[:, ch * C + ct * 128: ch * C + (ct + 1) * 128],
                                sb_featbf[:, ch * N + oc4 * 1024 + half * OC:
                                          ch * N + oc4 * 1024 + (half + 1) * OC],
                                start=(ch == 0), stop=(ch == 1))
                    kslice = kp[:, ct * N + oc4 * 1024: ct * N + (oc4 + 1) * 1024]
                    if ct == 0:
                        nc.vector.tensor_copy(kslice, psk[:])
                    else:
                        nc.scalar.activation(kslice, psk[:], ACTF.Copy)
                psq = psA.tile([128, 2 * OC], FP32, tag="s", name="psq")
                for half in range(2):
                    for ch in range(2):
                        nc.tensor.matmul(
                            psq[:, half * OC:(half + 1) * OC],
                            sb_wq[:, ch * C + ct * 128: ch * C + (ct + 1) * 128],
                            sb_featq[:, ch * O + half * OC:
                                     ch * O + (half + 1) * OC],
                            start=(ch == 0), stop=(ch == 1))
                nc.vector.tensor_scalar_add(
                    qp[:, ct * O:(ct + 1) * O], psq[:], sb_bq2[:, ct:ct + 1])

            # ---------------- attention (software-pipelined) ----------------
            msg_sb = cpool.tile([128, 2 * O], FP32, tag="msg")
            pvs = {}      # (oc, ct) -> pv psum tile
            x4s = {}      # oc -> conv1 input [64, 4*OC]
            cc_outs = {}

            def emit_qk(oc, ct, it):
                oco = oc * OC
                sps = psA.tile([128, 2 * OC], FP32, tag="s", name="sps")
                nc.tensor.matmul(
                    sps[:, 0:OC],
                    kp[0:64, ct * N + it * 128: ct * N + (it + 1) * 128],
                    qp[0:64, ct * O + oco: ct * O + oco + OC],
                    start=True, stop=True, tile_position=(0, 0))
                nc.tensor.matmul(
                    sps[:, OC:2 * OC],
                    kp[64:128, ct * N + it * 128: ct * N + (it + 1) * 128],
                    qp[64:128, ct * O + oco: ct * O + oco + OC],
                    start=True, stop=True, tile_position=(64, 0))
                return sps

            uctr = [0]

            def emit_expv(oc, ct, it, sps):
                if (oc, ct) not in pvs:
                    pvs[(oc, ct)] = psB.tile([128, 2 * OC], FP32, tag="pv",
                                             name=f"pv{oc}{ct}")
                pvp = pvs[(oc, ct)]
                uctr[0] += 1
                # first 12 units ScalarE-only: DVE is still draining the
                # projection/V^T eviction queue at kernel start.
                if uctr[0] <= 12 or it % 8 in SC_PAT:
                    et = epool.tile([128, 2 * OC], BF16, tag="et", name="et")
                    nc.scalar.activation(et[:], sps[:], ACTF.Exp, scale=0.125)
                    el, er = et[:, 0:OC], et[:, OC:2 * OC]
                else:
                    eti = epool.tile([128, 2 * OC], I16, tag="eti", name="eti")
                    nc.vector.tensor_scalar(
                        eti[:], sps[:], SCH_A, SCH_B, ALU.mult, ALU.add)
                    el = eti[:, 0:OC].bitcast(BF16)
                    er = eti[:, OC:2 * OC].bitcast(BF16)
                nc.tensor.matmul(
                    pvp[0:65, 0:OC],
                    vt[:, it * 260 + 2 * ct * 65: it * 260 + 2 * ct * 65 + 65],
                    el, start=(it == 0), stop=(it == NT - 1))
                nc.tensor.matmul(
                    pvp[0:65, OC:2 * OC],
                    vt[:, it * 260 + (2 * ct + 1) * 65:
                       it * 260 + (2 * ct + 1) * 65 + 65],
                    er, start=(it == 0), stop=(it == NT - 1))
                if it == NT - 1:
                    norm_pair(oc, ct)

            def norm_pair(oc, ct):
                """Row sums -> reciprocal -> broadcast -> x = featbv - M*r."""
                pvp = pvs[(oc, ct)]
                sp = npool.tile([65, 2 * OC], FP32, tag="sp", name="sp")
                nc.vector.tensor_copy(sp[64:65, :], pvp[64:65, :])
                s2 = npool.tile([2, OC], FP32, tag="s2", name="s2")
                nc.sync.dma_start(s2[:], sp[64:65, :])
                r2 = npool.tile([2, OC], FP32, tag="r2", name="r2")
                nc.vector.reciprocal_approx_fast(r2[:], s2[:])
                rf = npool.tile([1, 2 * OC], FP32, tag="rf", name="rf")
                nc.sync.dma_start(rf[:], r2[:])
                rb = npool.tile([64, 2 * OC], FP32, tag="rb", name="rb")
                nc.gpsimd.partition_broadcast(rb[:], rf[0:1, :])
                mr = wpool.tile([64, 2 * OC], BF16, tag="mr", name="mr")
                nc.vector.tensor_tensor(mr[:], pvp[0:64, :], rb[:], ALU.mult)
                if oc not in x4s:
                    x4s[oc] = wpool.tile([64, 4 * OC], BF16, tag="x4",
                                         name=f"x4_{oc}")
                nc.vector.tensor_tensor(
                    x4s[oc][:, ct * 2 * OC:(ct + 1) * 2 * OC]
                    .rearrange("p (h q) -> p h q", h=2),
                    sb_featbv4[:, 2 * ct * O:(2 * ct + 2) * O]
                    .rearrange("p (h o) -> p h o", h=2)[
                        :, :, oc * OC:oc * OC + OC],
                    mr[:].rearrange("p (h q) -> p h q", h=2),
                    ALU.subtract)

            def attn_seq(oc, unit_list, pending):
                for (ct, it) in unit_list:
                    sps = emit_qk(oc, ct, it)
                    if pending is not None:
                        emit_expv(*pending)
                    pending = (oc, ct, it, sps)
                return pending

            def flush(pending):
                if pending is not None:
                    emit_expv(*pending)
                return None

            def conv_chunk(oc):
                oco = oc * OC
                x4 = x4s[oc]
                ps1 = psA.tile([128, 2 * OC], FP32, tag="s", name="ps1")
                for h in range(4):
                    nc.tensor.matmul(
                        ps1[:, 0:OC], sb_w14[:, h * CH:(h + 1) * CH],
                        x4[:, h * OC:(h + 1) * OC],
                        start=(h == 0), stop=(h == 3))
                h1 = wpool.tile([128, OC], BF16, tag="h1", name="h1")
                nc.scalar.activation(h1[:], ps1[:, 0:OC], ACTF.Relu,
                                     bias=sb_bn1b[:, 0:1])
                ps2 = psA.tile([128, 2 * OC], FP32, tag="s", name="ps2")
                nc.tensor.matmul(ps2[:, 0:OC], sb_w2[:], h1[:],
                                 start=True, stop=True)
                h2 = wpool.tile([128, OC], BF16, tag="h2", name="h2")
                h2s = cpool.tile([128, 1], FP32, tag=f"h2s{oc}",
                                 name=f"h2s{oc}")
                nc.scalar.activation(h2[:], ps2[:, 0:OC], ACTF.Relu,
                                     bias=sb_bn2b[:, 0:1], accum_out=h2s[:])
                ps3 = psA.tile([128, 2 * OC], FP32, tag="s", name="ps3")
                for ct in range(2):
                    nc.tensor.matmul(
                        ps3[:, ct * OC:(ct + 1) * OC],
                        sb_w3[:, ct * 128:(ct + 1) * 128], h2[:],
                        start=True, stop=True, skip_group_check=True)
                for ct in range(2):
                    nc.vector.tensor_scalar_add(
                        msg_sb[:, ct * O + oco: ct * O + oco + OC],
                        ps3[:, ct * OC:(ct + 1) * OC], sb_b32[:, ct:ct + 1])
                # SE squeeze partial: AllGather h2 column-sums across 4 cores
                cc_in = dpool.tile([128, 1], FP32, tag=f"cci{oc}",
                                   name=f"cci{oc}")
                cc_out = dpool.tile([512, 1], FP32, tag=f"cco{oc}",
                                    name=f"cco{oc}")
                cc_outs[oc] = cc_out
                nc.sync.dma_start(cc_in[:], h2s[:])
                nc.gpsimd.collective_compute(
                    "AllGather", ALU.bypass,
                    replica_groups=[[0, 1, 2, 3], [4, 5, 6, 7]],
                    ins=[cc_in.opt()], outs=[cc_out.opt()])

            # emission: chunk0 | chunk1 first part | conv0+cc | chunk1 rest |
            # conv1+cc | tail.  The first collective overlaps chunk-1 attn.
            units = [(ct, it) for ct in range(2) for it in range(NT)]
            p = attn_seq(0, units, None)
            p = flush(p)
            p = attn_seq(1, units[:24], None)
            p = flush(p)
            conv_chunk(0)
            p = attn_seq(1, units[24:], None)
            p = flush(p)
            conv_chunk(1)

            # ---------------- SE gate tail ----------------
            sq_g = wpool.tile([128, 8], FP32, tag="sqg")
            for oc in range(2):
                nc.sync.dma_start(
                    sq_g[:, 4 * oc:4 * oc + 4],
                    cc_outs[oc][:].rearrange("(s p) k -> p (s k)", p=128))
            sq_t = wpool.tile([128, 4], FP32, tag="sqt")
            nc.vector.tensor_tensor(sq_t[:], sq_g[:, 0:4], sq_g[:, 4:8],
                                    ALU.add)
            sq_t2 = wpool.tile([128, 2], FP32, tag="sqt2")
            nc.vector.tensor_tensor(sq_t2[:], sq_t[:, 0:2], sq_t[:, 2:4],
                                    ALU.add)
            hs_bf = wpool.tile([128, 1], BF16, tag="hsbf")
            nc.vector.tensor_tensor(hs_bf[:], sq_t2[:, 0:1], sq_t2[:, 1:2],
                                    ALU.add)

            fc_ps = psA.tile([128, 2 * OC], FP32, tag="s", name="fc_ps")
            nc.tensor.matmul(fc_ps[0:16, 0:1], sb_wsq[:, 0:16], hs_bf[:],
                             start=True, stop=True)
            fc_sb = wpool.tile([16, 1], BF16, tag="fc")
            nc.vector.tensor_scalar(fc_sb[:], fc_ps[0:16, 0:1],
                                    sb_bsq[:, 0:1], 0.0, ALU.add, ALU.max)

            g_ps = psB.tile([128, 2 * OC], FP32, tag="pv", name="g_ps")
            for ct in range(2):
                nc.tensor.matmul(g_ps[:, ct:ct + 1],
                                 sb_wse2[:, ct * 128:(ct + 1) * 128],
                                 fc_sb[:], start=True, stop=True,
                                 skip_group_check=True)
            # sigmoid(x) = 1/(1+exp(-x)); bse2 negated on host
            ge = wpool.tile([128, 2], FP32, tag="ge")
            for ct in range(2):
                nc.scalar.activation(ge[:, ct:ct + 1], g_ps[:, ct:ct + 1],
                                     ACTF.Exp, bias=sb_bse22[:, ct:ct + 1],
                                     scale=-1.0)
            nc.vector.tensor_scalar_add(ge[:], ge[:], 1.0)
            gate = wpool.tile([128, 2], FP32, tag="gate")
            nc.vector.reciprocal_approx_fast(gate[:], ge[:])

            # out = feat_res + msg * gate
            for ct in range(2):
                nc.vector.scalar_tensor_tensor(
                    out=msg_sb[:, ct * O:(ct + 1) * O],
                    in0=msg_sb[:, ct * O:(ct + 1) * O],
                    scalar=gate[:, ct:ct + 1],
                    in1=sb_featres[:, ct * O:(ct + 1) * O],
                    op0=ALU.mult, op1=ALU.add)
                nc.sync.dma_start(out_d[ct * 128:(ct + 1) * 128, :],
                                  msg_sb[:, ct * O:(ct + 1) * O])

    nc.compile()
    return nc


def _prep_inputs(inputs):
    bf = ml_dtypes.bfloat16
    f = lambda x: np.ascontiguousarray(np.asarray(x, dtype=np.float32))
    feat = f(inputs["feat"])
    Wq, Wk, Wv = f(inputs["Wq"]), f(inputs["Wk"]), f(inputs["Wv"])
    bq, bv = f(inputs["bq"]), f(inputs["bv"])
    W1, W2, W3 = f(inputs["W1"]), f(inputs["W2"]), f(inputs["W3"])
    b1, b2, b3 = f(inputs["b1"]), f(inputs["b2"]), f(inputs["b3"])
    g1, be1, m1, v1 = f(inputs["g1"]), f(inputs["be1"]), f(inputs["m1"]), f(inputs["v1"])
    g2, be2, m2, v2 = f(inputs["g2"]), f(inputs["be2"]), f(inputs["m2"]), f(inputs["v2"])
    Wse1, Wse2 = f(inputs["Wse1"]), f(inputs["Wse2"])
    bse1, bse2 = f(inputs["bse1"]), f(inputs["bse2"])

    s1 = g1 / np.sqrt(v1 + EPS)
    sh1 = be1 - m1 * s1 + b1 * s1
    W1p = W1 * s1[:, None]
    s2 = g2 / np.sqrt(v2 + EPS)
    sh2 = be2 - m2 * s2 + b2 * s2
    W2p = W2 * s2[:, None]

    w1_4 = np.concatenate(
        [np.ascontiguousarray(W1p[:, 64 * h:64 * h + 64].T) for h in range(4)],
        axis=1)                                            # [64, 4*128]
    wsq = (Wse1 @ W3) / np.float32(N)                      # [16, 128]
    bsqv = Wse1 @ b3 + bse1                                # [16]

    common = {
        "wq_t": np.ascontiguousarray(Wq.T).astype(bf),
        "wk_t": np.ascontiguousarray(Wk.T).astype(bf),
        "wv_t": np.ascontiguousarray(Wv.T).astype(bf),
        "bq2": np.ascontiguousarray(bq.reshape(2, 128).T),
        "w1_4": np.ascontiguousarray(w1_4).astype(bf),
        "w2_t": np.ascontiguousarray(W2p.T).astype(bf),
        "w3_t": np.ascontiguousarray(W3.T).astype(bf),
        "bn1_b": sh1.reshape(128, 1),
        "bn2_b": sh2.reshape(128, 1),
        "b3_2": np.ascontiguousarray(b3.reshape(2, 128).T),
        "wsq_t": np.ascontiguousarray(wsq.T).astype(bf),
        "bsq": bsqv.reshape(16, 1),
        "wse2_t": np.ascontiguousarray(Wse2.T).astype(bf),
        "bse2_2": np.ascontiguousarray((-bse2).reshape(2, 128).T),
    }

    in_maps = []
    for core in range(8):
        b, osl = core // 4, core % 4
        o0 = osl * O
        fb = feat[b]
        m = dict(common)
        m["feat_bf"] = fb.astype(bf)
        m["feat_q"] = np.ascontiguousarray(fb[:, o0:o0 + O]).astype(bf)
        m["feat_res"] = np.ascontiguousarray(fb[:, o0:o0 + O])
        fbv = fb[:, o0:o0 + O] - bv[:, None]
        m["feat_bv4"] = np.ascontiguousarray(
            np.concatenate([fbv[64 * h:64 * h + 64, :] for h in range(4)],
                           axis=1)).astype(bf)
        in_maps.append(m)
    return in_maps


def kernel(**inputs) -> np.ndarray:
    if "nc" not in _CACHE:
        _CACHE["nc"] = _build()
    nc = _CACHE["nc"]
    in_maps = _prep_inputs(inputs)
    res = run_bass_kernel_spmd(nc, in_maps, core_ids=list(range(8)))
    out = np.zeros((BS, C, N), dtype=np.float32)
    for core in range(8):
        b, osl = core // 4, core % 4
        out[b, :, osl * O:(osl + 1) * O] = res.results[core]["out"]
    return out


if __name__ == "__main__":
    import sys
    sys.path.insert(0, "/root/problem")
    from reference import setup_inputs, reference
    inp = {k: np.asarray(v) for k, v in setup_inputs().items()}
    ref = np.asarray(reference(**inp))
    got = kernel(**inp)
    err = np.abs(got - ref)
    print("absmax err:", err.max(), "ref absmax:", np.abs(ref).max())
    print("Relative error:", err.max() / np.abs(ref).max())
